# revision 6
# baseline (speedup 1.0000x reference)
"""ChoiceAttention Trainium2 kernel.

Math (per batch item b, per "retain" iteration a over the 5 options):
    q_a = opt_a @ W                              (s, h)
    S_ak[p, r] = q_a[p, :] . opt_k[r, :]         for the 4 options k != a
    w_ak = softmax over k of (S_ak + bias)       (bias cancels: softmax is
                                                  shift-invariant over k)
    out += sum_k w_ak @ opt_k
final out /= 2.

Sharding: data-parallel over batch across 8 NeuronCores (4 items each),
W replicated. No collectives; host concatenates the per-core outputs.

Layout strategy per core / batch item:
    nat_k : opt_k natural layout      (128p, 2 sc, 1024h)  - DMA'd in
    x_k   : opt_k transposed (h-major)(128p, 8 hc, 256s)   - PE transposes
    q_a^T : h-major q                 (128p, 8 hc, 256s)   - matmul(W, x_a)
    S_ak^T: scores transposed         (128p, 2 rc, 256p)   - matmul(x_k, q_a^T)
    softmax over the four k tiles elementwise (max-subtract, exp, recip)
    out   : accumulated in 4 PSUM banks over all 40 (a,k,rc) matmul groups
All matmuls run as float32r (full PE rate, fp32 storage).
"""

import numpy as np

B, S, H = 32, 256, 1024
NCORES = 8
BPC = B // NCORES  # batch items per core
P = 128
HC = H // P  # 8 h-chunks
SC = S // P  # 2 s-chunks
NOPT = 5

_CACHE: dict = {}


def _build_bass(reps: int = 1):
    from contextlib import ExitStack

    import concourse.mybir as mybir
    import concourse.tile as tile
    from concourse import bacc
    from concourse.masks import make_identity

    FP32 = mybir.dt.float32
    F32R = mybir.dt.float32r
    AF = mybir.ActivationFunctionType

    nc = bacc.Bacc(debug=False)

    opt_d = [
        nc.dram_tensor(f"option{i + 1}", (BPC, S, H), F32R, kind="ExternalInput")
        for i in range(NOPT)
    ]
    w_d = nc.dram_tensor("W", (H, H), F32R, kind="ExternalInput")
    out_d = nc.dram_tensor("out", (BPC, S, H), FP32, kind="ExternalOutput")

    with ExitStack() as ctx:
        tc = ctx.enter_context(tile.TileContext(nc))
        const = ctx.enter_context(tc.tile_pool(name="const", bufs=1))
        natp = ctx.enter_context(tc.tile_pool(name="nat", bufs=NOPT))
        xp = ctx.enter_context(tc.tile_pool(name="xt", bufs=NOPT))
        qp = ctx.enter_context(tc.tile_pool(name="qq", bufs=3))
        sp = ctx.enter_context(tc.tile_pool(name="ss", bufs=6))
        ep = ctx.enter_context(tc.tile_pool(name="ee", bufs=6))
        mp_ = ctx.enter_context(tc.tile_pool(name="mm", bufs=2))
        zp = ctx.enter_context(tc.tile_pool(name="zz", bufs=2))
        rp = ctx.enter_context(tc.tile_pool(name="rr", bufs=2))
        op_ = ctx.enter_context(tc.tile_pool(name="osb", bufs=2))
        ps_misc = ctx.enter_context(tc.tile_pool(name="ps_misc", bufs=2, space="PSUM"))
        ps_s = ctx.enter_context(tc.tile_pool(name="ps_s", bufs=2, space="PSUM"))
        ps_o = ctx.enter_context(tc.tile_pool(name="ps_o", bufs=4, space="PSUM"))

        ident_f = const.tile([P, P], FP32)
        make_identity(nc, ident_f)
        ident = const.tile([P, P], F32R)
        nc.vector.tensor_copy(out=ident, in_=ident_f)
        w_sb = const.tile([P, HC, H], F32R)
        nc.sync.dma_start(out=w_sb, in_=w_d.ap().rearrange("(kc p) h -> p kc h", p=P))

        from contextlib import nullcontext

        loop_cm = tc.For_i(0, reps, 1) if reps > 1 else nullcontext()
        with loop_cm:
         for b in range(BPC):
            # ---- load natural layout, build h-major transposed copies ----
            nat = []
            for k in range(NOPT):
                nk = natp.tile([P, SC, H], F32R, tag="nat")
                nc.sync.dma_start(
                    out=nk, in_=opt_d[k].ap()[b].rearrange("(sc p) h -> p sc h", p=P)
                )
                nat.append(nk)

            x = []
            for k in range(NOPT):
                xk = xp.tile([P, HC, S], F32R, tag="xt")
                for j in range(HC // 2):  # pairs of h-chunks -> one PSUM bank
                    pt = ps_misc.tile([P, 4, P], F32R, tag="ps_misc")
                    for d in range(2):
                        hc = 2 * j + d
                        for sc in range(SC):
                            nc.tensor.transpose(
                                out=pt[:, 2 * d + sc, :],
                                in_=nat[k][:, sc, hc * P : (hc + 1) * P],
                                identity=ident,
                            )
                    dst = xk[:, 2 * j : 2 * j + 2, :]
                    if (k + j) % 2 == 0:
                        nc.scalar.copy(out=dst, in_=pt)
                    else:
                        nc.vector.tensor_copy(out=dst, in_=pt)
                x.append(xk)

            # ---- q_a^T = W^T @ opt_a^T, pipelined with the a-loop ----
            q = [None] * NOPT

            def emit_q(a):
                qt = qp.tile([P, HC, S], F32R, tag="qq")
                for half in range(HC // 2):  # pairs of output h-chunks
                    pq = ps_misc.tile([P, 2, S], FP32, tag="ps_misc")
                    for d in range(2):
                        mc = 2 * half + d
                        for kc in range(HC):
                            nc.tensor.matmul(
                                pq[:, d, :],
                                w_sb[:, kc, mc * P : (mc + 1) * P],
                                x[a][:, kc, :],
                                start=(kc == 0),
                                stop=(kc == HC - 1),
                            )
                    nc.scalar.copy(out=qt[:, 2 * half : 2 * half + 2, :], in_=pq)
                q[a] = qt

            # scores S_ak^T (r-major) for all k != a, evacuated to SBUF
            def emit_scores(a):
                s_sb = []
                for k in range(NOPT):
                    if k == a:
                        continue
                    st = ps_s.tile([P, SC, S], FP32, tag="ps_s")
                    for rc in range(SC):
                        for hc in range(HC):
                            nc.tensor.matmul(
                                st[:, rc, :],
                                x[k][:, hc, rc * P : (rc + 1) * P],
                                q[a][:, hc, :],
                                start=(hc == 0),
                                stop=(hc == HC - 1),
                            )
                    ssb = sp.tile([P, SC, S], FP32, tag="ss")
                    nc.scalar.copy(out=ssb, in_=st)
                    s_sb.append(ssb)
                return s_sb

            def emit_softmax(a, s_sb):
                # max over the 4 option tiles, subtract, exp, normalize
                m = mp_.tile([P, SC, S], FP32, tag="mm")
                nc.vector.tensor_max(m, s_sb[0], s_sb[1])
                nc.vector.tensor_max(m, m, s_sb[2])
                nc.vector.tensor_max(m, m, s_sb[3])
                e = []
                for k4 in range(4):
                    nc.vector.tensor_sub(s_sb[k4], s_sb[k4], m)
                    ek = ep.tile([P, SC, S], F32R, tag="ee")
                    nc.scalar.activation(out=ek, in_=s_sb[k4], func=AF.Exp)
                    e.append(ek)
                z = zp.tile([P, SC, S], FP32, tag="zz")
                rcp = rp.tile([P, SC, S], FP32, tag="rr")
                nc.vector.tensor_add(z, e[0], e[1])
                nc.vector.tensor_add(rcp, e[2], e[3])
                nc.vector.tensor_add(z, z, rcp)
                nc.vector.reciprocal(rcp, z)
                for k4 in range(4):
                    nc.vector.tensor_mul(e[k4], e[k4], rcp)
                return e  # softmax weights, transposed layout (r, p)

            po = [[ps_o.tile([P, 512], FP32, tag="ps_o", name=f"po_{b}_{i}_{j}")
                   for j in range(2)] for i in range(SC)]
            po_started = [[False] * 2 for _ in range(SC)]

            def emit_out(a, w4, last):
                ks = [k for k in range(NOPT) if k != a]
                for i4, k in enumerate(ks):
                    for mp2 in range(SC):
                        for nn in range(2):
                            for rc in range(SC):
                                is_last = last and i4 == 3 and rc == SC - 1
                                nc.tensor.matmul(
                                    po[mp2][nn],
                                    w4[i4][:, rc, mp2 * P : (mp2 + 1) * P],
                                    nat[k][:, rc, nn * 512 : (nn + 1) * 512],
                                    start=(not po_started[mp2][nn]),
                                    stop=is_last,
                                )
                                po_started[mp2][nn] = True

            # software-pipelined a-loop: PE always has score/q matmuls queued
            # while the previous iteration's softmax runs on ACT/DVE.
            emit_q(0)
            emit_q(1)
            s_cur = emit_scores(0)
            for a in range(NOPT):
                if a + 2 < NOPT:
                    emit_q(a + 2)
                w4 = emit_softmax(a, s_cur)
                if a + 1 < NOPT:
                    s_cur = emit_scores(a + 1)
                emit_out(a, w4, last=(a == NOPT - 1))

            # ---- evacuate, fold the /2, store ----
            osb = op_.tile([P, SC, H], FP32, tag="osb")
            for mp2 in range(SC):
                for nn in range(2):
                    nc.scalar.activation(
                        out=osb[:, mp2, nn * 512 : (nn + 1) * 512],
                        in_=po[mp2][nn],
                        func=AF.Copy,
                        scale=0.5,
                    )
            nc.sync.dma_start(
                out=out_d.ap()[b].rearrange("(sc p) h -> p sc h", p=P), in_=osb
            )

    nc.compile()
    return nc


def _get_nc(reps: int = 1):
    key = f"nc{reps}"
    if key not in _CACHE:
        _CACHE[key] = _build_bass(reps)
    return _CACHE[key]


def kernel(**inputs) -> np.ndarray:
    from concourse.bass_utils import run_bass_kernel_spmd

    nc = _get_nc()
    opts = [np.ascontiguousarray(np.asarray(inputs[f"option{i + 1}"], dtype=np.float32))
            for i in range(NOPT)]
    W = np.ascontiguousarray(np.asarray(inputs["W"], dtype=np.float32))

    in_maps = []
    for c in range(NCORES):
        m = {f"option{i + 1}": opts[i][c * BPC : (c + 1) * BPC] for i in range(NOPT)}
        m["W"] = W
        in_maps.append(m)

    res = run_bass_kernel_spmd(nc, in_maps, list(range(NCORES)))
    out = np.concatenate([res.results[c]["out"] for c in range(NCORES)], axis=0)
    return np.asarray(out, dtype=np.float32)
